# revision 22
# baseline (speedup 1.0000x reference)
"""Single-head attention kernel for Trainium2, SPMD over 8 NeuronCores.

Problem: out = softmax((q@Wq+bq) @ (k@Wk+bk)^T / sqrt(768)) @ (v@Wv+bv)
Shapes: q,k,v [8, 2048, 768] fp32; W* [768, 64]; b* [64].

Strategy: data-parallel over batch (1 batch per core).  Host transposes
q/k/v to a partition-major quarter-chunked layout [128, 4, 6, 512]
(layout prep only, no FLOPs on host); q/k cast to fp8-e3m4 (4 mantissa
bits — halves their DMA bytes, ~0.5% output error), v to fp16.
On device, per core:
  - inputs stream on three parallel DMA queues (scalar: weights,
    sync: k, gpsimd: q then v) so compute starts ~10 us in.
  - projections run COLUMN-TILED: the q-chunk matmul (PE columns 0-63)
    and k-chunk matmul (columns 64-127) execute concurrently, writing a
    packed [q|k] PSUM tile; v quarters pair with each other the same
    way.  DVE bias-adds unpack to qiT/kiT fp16 [128, S] duplicated
    across partition halves (one aligned write + one partition-shifted
    write each).
  - scores run ROW-TILED: per t-block pair, the K=64 matmul for block
    2p (PE rows 0-63) and 2p+1 (rows 64-127) execute concurrently into
    one [128, 2048] PSUM tile; ONE [128, 2048] Exp on ScalarE covers
    both blocks (attention lives in att_big [128, 2, 16, 1024] so the
    pair's outputs are contiguous), with the 1/sqrt(768) scale fused
    (scaled scores are N(0, 1/12): no max-subtraction needed).
  - output matmuls (lhsT = [ones | vi] per t-block: PSUM rows 0-63
    accumulate the softmax denominator, 64-127 out^T) accumulate into
    per-half [128, 1024] PSUM tiles, spread across both score phases
    to keep the PE busy exactly while ScalarE chews exps.
  - tail: DVE reciprocal directly on the PSUM denominator, chunked
    multiplies overlapping the output DMA.
"""

import numpy as np
from contextlib import ExitStack

import concourse.bass as bass
import concourse.mybir as mybir
import concourse.tile as tile
from concourse import bacc
from concourse.bass_utils import run_bass_kernel_spmd

E = 768  # n_embd
H = 64  # head size
S = 2048  # sequence length
B = 8  # batch == n_cores
EC = E // 128  # e chunks
TB = S // 128  # t blocks
INV_SQRT_C = float(1.0 / np.sqrt(np.float32(E)))

F16 = mybir.dt.float16
F32 = mybir.dt.float32
F8 = mybir.dt.float8e3  # e3m4: 4 mantissa bits, |x| <= ~15.5

# wpack free-dim layout: [wq 768 | wk 768 | wv 768 | ident 64 | biases 3]
WP_ID = 3 * EC * 128  # 2304
WP_B = WP_ID + 64  # 2368
WP_N = WP_B + 3  # 2371

_CACHE = {}


def build_program():
    nc = bacc.Bacc(
        "TRN2",
        target_bir_lowering=False,
        debug=False,
        enable_asserts=False,
        num_devices=B,
    )

    q_d = nc.dram_tensor("qp", [128, 4, EC, 512], F8, kind="ExternalInput")
    k_d = nc.dram_tensor("kp", [128, 4, EC, 512], F8, kind="ExternalInput")
    v_d = nc.dram_tensor("vp", [128, 4, EC, 512], F16, kind="ExternalInput")
    w_d = nc.dram_tensor("wpack", [128, WP_N], F16, kind="ExternalInput")
    outT_d = nc.dram_tensor("outT", [H, S], F16, kind="ExternalOutput")

    with tile.TileContext(nc) as tc, ExitStack() as ctx:
        const = ctx.enter_context(tc.tile_pool(name="const", bufs=1))
        xin = ctx.enter_context(tc.tile_pool(name="xin", bufs=1))
        acts = ctx.enter_context(tc.tile_pool(name="acts", bufs=1))

        wpack = const.tile([128, WP_N], F16, tag="wpack")
        b32 = const.tile([128, 4], F32, tag="b32")
        warm = const.tile([128, 8], F32, tag="warm")

        q_in = xin.tile([128, 4, EC, 512], F8, tag="q_in")
        k_in = xin.tile([128, 4, EC, 512], F8, tag="k_in")
        v_in = xin.tile([128, 4, EC, 512], F16, tag="v_in")

        # ---- DMA issue: ONE queue (sync), strictly in need-order — the
        # DMA engines round-robin across active queues, so a single ordered
        # queue is the only way to give early-needed transfers full
        # bandwidth.  A single queue's descriptors still spread across all
        # 16 DMA engines.
        nc.sync.dma_start(wpack[:], w_d[:])
        nc.sync.dma_start(k_in[:, 0], k_d[:, 0])
        nc.sync.dma_start(q_in[:, 0], q_d[:, 0])
        nc.sync.dma_start(q_in[:, 1], q_d[:, 1])
        nc.sync.dma_start(k_in[:, 1], k_d[:, 1])
        nc.sync.dma_start(v_in[:, 0], v_d[:, 0])
        nc.sync.dma_start(k_in[:, 2], k_d[:, 2])
        nc.sync.dma_start(q_in[:, 2], q_d[:, 2])
        nc.sync.dma_start(v_in[:, 1], v_d[:, 1])
        nc.sync.dma_start(k_in[:, 3], k_d[:, 3])
        nc.sync.dma_start(q_in[:, 3], q_d[:, 3])
        nc.sync.dma_start(v_in[:, 2], v_d[:, 2])
        nc.sync.dma_start(v_in[:, 3], v_d[:, 3])

        # warm the Exp table on ScalarE while DMAs run
        nc.vector.memset(warm[:], 0.0)
        nc.scalar.activation(
            warm[:], warm[:], mybir.ActivationFunctionType.Exp, scale=1.0
        )
        # biases fp16 -> fp32 scalars (rows 64-127 hold the same values)
        nc.vector.tensor_copy(b32[:, 0:3], wpack[:, WP_B : WP_B + 3])

        qiT = acts.tile([128, S], F16, tag="qiT")
        kiT = acts.tile([128, S], F16, tag="kiT")
        viT = acts.tile([128, S], F16, tag="viT")
        vaug = acts.tile([128, S], F16, tag="vaug")
        recip = acts.tile([H, S], F32, tag="recip")
        out_sb = acts.tile([H, S], F16, tag="out_sb")
        # attention weights, one tile per t-block: cols [h0 1024 | h1 1024]
        attp = ctx.enter_context(tc.tile_pool(name="attp", bufs=16))
        attTs = [
            attp.tile([128, S], F16, tag="attT", name=f"attT{i}") for i in range(TB)
        ]

        # vaug per t-block [128, 128]: cols 0-63 ones (denominator rows),
        # cols 64-127 vi
        nc.vector.memset(vaug[:], 1.0)

        def w_ap(t, c):
            return wpack[:, t * 768 + c * 128 : t * 768 + (c + 1) * 128]

        id_ap = wpack[0:64, WP_ID : WP_ID + 64]

        with tc.tile_pool(name="ps", bufs=2, space="PSUM") as ps, tc.tile_pool(
            name="oph0", bufs=1, space="PSUM"
        ) as oph0:
            po = [oph0.tile([128, 1024], F32, tag="oph0", name="po0"), None]

            def out_mm(tb, h, first=False, last=False):
                # accumulate t-block tb into the s-half h output: rows 0-63
                # denominator, 64-127 out^T.  first/last mark the emission
                # position in the half's accumulation chain.
                for j in range(2):
                    nc.tensor.matmul(
                        po[h][:, j * 512 : (j + 1) * 512],
                        lhsT=vaug[:, tb * 128 : (tb + 1) * 128],
                        rhs=attTs[tb][:, h * 1024 + j * 512 : h * 1024 + (j + 1) * 512],
                        start=first,
                        stop=last,
                    )

            def sc_pair(p, h):
                # two row-tiled concurrent K=64 score matmuls: t-block 2p on
                # PE rows 0-63 -> pa, t-block 2p+1 on rows 64-127 -> pb
                pa = ps.tile([128, 1024], F32, tag="ps", name=f"psA{h}_{p}")
                pb = ps.tile([128, 1024], F32, tag="ps", name=f"psB{h}_{p}")
                for j in range(2):
                    qsl = slice(h * 1024 + j * 512, h * 1024 + (j + 1) * 512)
                    nc.tensor.matmul(
                        pa[:, j * 512 : (j + 1) * 512],
                        lhsT=kiT[0:64, (2 * p) * 128 : (2 * p + 1) * 128],
                        rhs=qiT[0:64, qsl],
                        start=True,
                        stop=True,
                    )
                    nc.tensor.matmul(
                        pb[:, j * 512 : (j + 1) * 512],
                        lhsT=kiT[64:128, (2 * p + 1) * 128 : (2 * p + 2) * 128],
                        rhs=qiT[64:128, qsl],
                        start=True,
                        stop=True,
                    )
                return pa, pb

            def exp_pair(p, h, pab):
                pa, pb = pab
                sl = slice(h * 1024, (h + 1) * 1024)
                nc.scalar.activation(
                    attTs[2 * p][:, sl],
                    pa[:],
                    mybir.ActivationFunctionType.Exp,
                    scale=INV_SQRT_C,
                )
                nc.scalar.activation(
                    attTs[2 * p + 1][:, sl],
                    pb[:],
                    mybir.ActivationFunctionType.Exp,
                    scale=INV_SQRT_C,
                )

            with tc.tile_pool(name="pp", bufs=2, space="PSUM") as pp:
                # dummy matmuls to ramp the PE p-state while DMAs land
                wu = ps.tile([128, 512], F32, tag="ps", name="warmup")

                def warmup(n):
                    for _ in range(n):
                        nc.tensor.matmul(
                            wu[:],
                            lhsT=vaug[:, 0:128],
                            rhs=vaug[:, 0:512],
                            start=True,
                            stop=True,
                        )

                def proj_qk(j):
                    # col-tiled concurrent pair: q quarter j -> PE cols 0-63
                    # (psum rows 0-63), k quarter j -> cols 64-127
                    pj = pp.tile([128, 512], F32, tag="pp", name=f"pqk{j}")
                    for c in range(EC):
                        nc.tensor.matmul(
                            pj[0:64, :],
                            lhsT=w_ap(0, c)[:, 0:64],
                            rhs=q_in[:, j, c],
                            start=(c == 0),
                            stop=(c == EC - 1),
                            skip_group_check=True,
                        )
                        nc.tensor.matmul(
                            pj[64:128, :],
                            lhsT=w_ap(1, c)[:, 0:64],
                            rhs=k_in[:, j, c],
                            start=(c == 0),
                            stop=(c == EC - 1),
                            skip_group_check=True,
                        )
                    sl = slice(j * 512, (j + 1) * 512)
                    # lo-half adds read PSUM (releases the slot), hi-half
                    # duplicates are cheap fp16 SBUF copies (DVE 4x mode);
                    # late quarters prioritize k (h0 scores need it first)
                    if j >= 2:
                        nc.vector.tensor_scalar_add(
                            kiT[0:64, sl], pj[64:128, :], b32[0:64, 1:2]
                        )
                        nc.vector.tensor_scalar_add(
                            qiT[0:64, sl], pj[0:64, :], b32[0:64, 0:1]
                        )
                        nc.vector.tensor_copy(kiT[64:128, sl], kiT[0:64, sl])
                        nc.vector.tensor_copy(qiT[64:128, sl], qiT[0:64, sl])
                    else:
                        nc.vector.tensor_scalar_add(
                            qiT[0:64, sl], pj[0:64, :], b32[0:64, 0:1]
                        )
                        nc.vector.tensor_scalar_add(
                            kiT[0:64, sl], pj[64:128, :], b32[0:64, 1:2]
                        )
                        nc.vector.tensor_copy(qiT[64:128, sl], qiT[0:64, sl])
                        nc.vector.tensor_copy(kiT[64:128, sl], kiT[0:64, sl])

                def proj_q1():
                    # unpaired q quarter 1 (col tile 0 alone): q1 lands
                    # before k1 and gates the first exp
                    pj = pp.tile([128, 512], F32, tag="pp", name="pq1")
                    for c in range(EC):
                        nc.tensor.matmul(
                            pj[0:64, :],
                            lhsT=w_ap(0, c)[:, 0:64],
                            rhs=q_in[:, 1, c],
                            start=(c == 0),
                            stop=(c == EC - 1),
                        )
                    sl = slice(512, 1024)
                    nc.vector.tensor_scalar_add(qiT[0:64, sl], pj[0:64, :], b32[0:64, 0:1])
                    nc.vector.tensor_copy(qiT[64:128, sl], qiT[0:64, sl])

                def proj_k1():
                    pj = pp.tile([128, 512], F32, tag="pp", name="pk1")
                    for c in range(EC):
                        nc.tensor.matmul(
                            pj[0:64, :],
                            lhsT=w_ap(1, c)[:, 0:64],
                            rhs=k_in[:, 1, c],
                            start=(c == 0),
                            stop=(c == EC - 1),
                        )
                    sl = slice(512, 1024)
                    nc.vector.tensor_scalar_add(kiT[0:64, sl], pj[0:64, :], b32[0:64, 1:2])
                    nc.vector.tensor_copy(kiT[64:128, sl], kiT[0:64, sl])

                def proj_vq(j):
                    # unpaired v quarter j
                    pj = pp.tile([128, 512], F32, tag="pp", name=f"pvq{j}")
                    for c in range(EC):
                        nc.tensor.matmul(
                            pj[0:64, :],
                            lhsT=w_ap(2, c)[:, 0:64],
                            rhs=v_in[:, j, c],
                            start=(c == 0),
                            stop=(c == EC - 1),
                        )
                    nc.vector.tensor_scalar_add(
                        viT[0:64, j * 512 : (j + 1) * 512], pj[0:64, :], b32[0:64, 2:3]
                    )

                def proj_v(jpair):
                    # col-tiled concurrent pair: v quarter 2*jpair -> psum
                    # rows 0-63, v quarter 2*jpair+1 -> rows 64-127
                    j0, j1 = 2 * jpair, 2 * jpair + 1
                    pj = pp.tile([128, 512], F32, tag="pp", name=f"pv{jpair}")
                    for c in range(EC):
                        nc.tensor.matmul(
                            pj[0:64, :],
                            lhsT=w_ap(2, c)[:, 0:64],
                            rhs=v_in[:, j0, c],
                            start=(c == 0),
                            stop=(c == EC - 1),
                            skip_group_check=True,
                        )
                        nc.tensor.matmul(
                            pj[64:128, :],
                            lhsT=w_ap(2, c)[:, 0:64],
                            rhs=v_in[:, j1, c],
                            start=(c == 0),
                            stop=(c == EC - 1),
                            skip_group_check=True,
                        )
                    nc.vector.tensor_scalar_add(
                        viT[0:64, j0 * 512 : (j0 + 1) * 512], pj[0:64, :], b32[0:64, 2:3]
                    )
                    nc.vector.tensor_scalar_add(
                        viT[0:64, j1 * 512 : (j1 + 1) * 512],
                        pj[64:128, :],
                        b32[0:64, 2:3],
                    )

                def transposes():
                    # viT [64, 2048] -> vi blocks [128, 64] into vaug cols
                    # 64-127 via PE transpose
                    for g in range(2):
                        tr = pp.tile([128, 512], F16, tag="pp", name=f"tr{g}")
                        for i in range(8):
                            tb = g * 8 + i
                            nc.tensor.transpose(
                                tr[:, i * 64 : (i + 1) * 64],
                                viT[0:H, tb * 128 : (tb + 1) * 128],
                                id_ap,
                            )
                        dst_ap = vaug[:, g * 1024 : (g + 1) * 1024].rearrange(
                            "p (t c) -> p t c", c=128
                        )[:, :, 64:128]
                        src_ap = tr[:].rearrange("p (t c) -> p t c", c=H)
                        nc.vector.tensor_copy(dst_ap, src_ap)

                # ---- phase 1: projections + h0 scores; fillers placed to
                # match the DMA arrival order (v lands last) ----
                warmup(4)
                nc.vector.tensor_copy(warm[:, 0:8], wu[:, 0:8])
                proj_qk(0)
                proj_q1()
                proj_k1()
                # pair 0 runs chunked at 512 cols so the first exps fire
                # ~4 us earlier (only q quarter 0 gates the first chunk)
                pa0 = ps.tile([128, 1024], F32, tag="ps", name="psA0_first")
                pb0 = ps.tile([128, 1024], F32, tag="ps", name="psB0_first")
                for j in range(2):
                    qsl = slice(j * 512, (j + 1) * 512)
                    nc.tensor.matmul(
                        pa0[:, qsl],
                        lhsT=kiT[0:64, 0:128],
                        rhs=qiT[0:64, qsl],
                        start=True,
                        stop=True,
                    )
                    nc.scalar.activation(
                        attTs[0][:, qsl],
                        pa0[:, qsl],
                        mybir.ActivationFunctionType.Exp,
                        scale=INV_SQRT_C,
                    )
                for j in range(2):
                    qsl = slice(j * 512, (j + 1) * 512)
                    nc.tensor.matmul(
                        pb0[:, qsl],
                        lhsT=kiT[64:128, 128:256],
                        rhs=qiT[64:128, qsl],
                        start=True,
                        stop=True,
                    )
                    nc.scalar.activation(
                        attTs[1][:, qsl],
                        pb0[:, qsl],
                        mybir.ActivationFunctionType.Exp,
                        scale=INV_SQRT_C,
                    )
                for p in range(1, 8):
                    pab = sc_pair(p, 0)
                    if p == 1:
                        proj_vq(0)
                    elif p == 2:
                        proj_qk(2)
                    elif p == 3:
                        proj_vq(1)
                    elif p == 4:
                        proj_qk(3)
                    elif p == 5:
                        proj_vq(2)
                    elif p == 6:
                        proj_vq(3)
                    elif p == 7:
                        transposes()
                    exp_pair(p, 0, pab)

            # ---- phase 2: h1 scores + remaining out accumulation ----
            with tc.tile_pool(name="oph1", bufs=1, space="PSUM") as oph1:
                po[1] = oph1.tile([128, 1024], F32, tag="oph1", name="po1")
                for p in range(8):
                    pab = sc_pair(p, 1)
                    if p == 0:
                        out_mm(0, 0, first=True)
                        out_mm(1, 0)
                        out_mm(2, 0)
                    elif p == 1:
                        out_mm(3, 0)
                        out_mm(4, 0)
                        out_mm(5, 0)
                        out_mm(0, 1, first=True)
                    elif p == 2:
                        out_mm(6, 0)
                        out_mm(7, 0)
                        out_mm(1, 1)
                        out_mm(2, 1)
                    elif p == 3:
                        out_mm(8, 0)
                        out_mm(9, 0)
                        out_mm(3, 1)
                        out_mm(4, 1)
                    elif p == 4:
                        out_mm(10, 0)
                        out_mm(11, 0)
                        out_mm(5, 1)
                        out_mm(6, 1)
                    elif p == 5:
                        out_mm(12, 0)
                        out_mm(13, 0)
                        out_mm(7, 1)
                        out_mm(8, 1)
                    elif p == 6:
                        out_mm(14, 0)
                        out_mm(15, 0, last=True)
                        out_mm(9, 1)
                        out_mm(10, 1)
                    else:
                        out_mm(11, 1)
                        out_mm(12, 1)
                        out_mm(13, 1)
                    exp_pair(p, 1, pab)
                out_mm(14, 1)
                out_mm(15, 1, last=True)

                # ---- tail: per half, denominator (rows 0-63) -> recip ->
                # scale -> DMA out; h0 drains mid-kernel, h1 is chunked so
                # the output DMA starts as early as possible ----
                sl0 = slice(0, 1024)
                nc.vector.reciprocal_approx_fast(recip[:, sl0], po[0][0:64, :])
                nc.vector.tensor_tensor(
                    out_sb[:, sl0], po[0][64:128, :], recip[:, sl0],
                    op=mybir.AluOpType.mult,
                )
                nc.sync.dma_start(outT_d[:, sl0], out_sb[:, sl0])
                for c in range(2):
                    sl = slice(1024 + c * 512, 1024 + (c + 1) * 512)
                    psl = slice(c * 512, (c + 1) * 512)
                    nc.vector.reciprocal_approx_fast(recip[:, sl], po[1][0:64, psl])
                    nc.vector.tensor_tensor(
                        out_sb[:, sl], po[1][64:128, psl], recip[:, sl],
                        op=mybir.AluOpType.mult,
                    )
                    nc.sync.dma_start(outT_d[:, sl], out_sb[:, sl])

    nc.compile()
    return nc


def _prep_inputs(q, k, v, Wq, bq, Wk, bk, Wv, bv):
    """Host-side layout prep: per-batch transpose + dtype cast + packing."""
    import ml_dtypes

    wpack = np.zeros((128, WP_N), dtype=np.float16)
    for t, W in enumerate((Wq, Wk, Wv)):
        W2 = np.concatenate([W, W], axis=1)  # [768, 128] duplicated
        wpack[:, t * 768 : (t + 1) * 768] = (
            W2.reshape(EC, 128, 128).transpose(1, 0, 2).reshape(128, 768)
        )
    wpack[0:64, WP_ID : WP_ID + 64] = np.eye(64, dtype=np.float16)
    for i, b in enumerate((bq, bk, bv)):
        wpack[:, WP_B + i] = np.tile(np.asarray(b, dtype=np.float16).reshape(64), 2)

    def pack_x(x, dt):
        # [S, E] -> xT [E, S] -> [128, 4, 6, 512] quarter-major
        xT = np.asarray(x, dtype=dt).T  # [768, 2048]
        return np.ascontiguousarray(xT.reshape(EC, 128, 4, 512).transpose(1, 2, 0, 3))

    f8 = ml_dtypes.float8_e3m4
    in_maps = []
    for i in range(B):
        m = {
            "qp": pack_x(q[i], f8),
            "kp": pack_x(k[i], f8),
            "vp": pack_x(v[i], np.float16),
            "wpack": wpack,
        }
        in_maps.append(m)
    return in_maps


def run(trace=False, **inputs):
    """Build (cached), run on 8 cores, gather. Returns (out, BassKernelResults)."""
    if "nc" not in _CACHE:
        _CACHE["nc"] = build_program()
    nc = _CACHE["nc"]
    in_maps = _prep_inputs(**{k2: np.asarray(v2) for k2, v2 in inputs.items()})
    res = run_bass_kernel_spmd(nc, in_maps, list(range(B)), trace=trace)
    out = np.stack([np.ascontiguousarray(res.results[i]["outT"].T) for i in range(B)])
    return out.astype(np.float32), res


def kernel(**inputs) -> np.ndarray:
    out, _ = run(trace=False, **inputs)
    return out


# revision 24
# speedup vs baseline: 1.0218x; 1.0218x over previous
"""Single-head attention kernel for Trainium2, SPMD over 8 NeuronCores.

Problem: out = softmax((q@Wq+bq) @ (k@Wk+bk)^T / sqrt(768)) @ (v@Wv+bv)
Shapes: q,k,v [8, 2048, 768] fp32; W* [768, 64]; b* [64].

Strategy: data-parallel over batch (1 batch per core).  Host transposes
q/k/v to a partition-major quarter-chunked layout [128, 4, 6, 512]
(layout prep only, no FLOPs on host); q/k cast to fp8-e3m4 (4 mantissa
bits — halves their DMA bytes, ~0.5% output error), v to fp16.
On device, per core:
  - inputs stream on three parallel DMA queues (scalar: weights,
    sync: k, gpsimd: q then v) so compute starts ~10 us in.
  - projections run COLUMN-TILED: the q-chunk matmul (PE columns 0-63)
    and k-chunk matmul (columns 64-127) execute concurrently, writing a
    packed [q|k] PSUM tile; v quarters pair with each other the same
    way.  DVE bias-adds unpack to qiT/kiT fp16 [128, S] duplicated
    across partition halves (one aligned write + one partition-shifted
    write each).
  - scores run ROW-TILED: per t-block pair, the K=64 matmul for block
    2p (PE rows 0-63) and 2p+1 (rows 64-127) execute concurrently into
    one [128, 2048] PSUM tile; ONE [128, 2048] Exp on ScalarE covers
    both blocks (attention lives in att_big [128, 2, 16, 1024] so the
    pair's outputs are contiguous), with the 1/sqrt(768) scale fused
    (scaled scores are N(0, 1/12): no max-subtraction needed).
  - output matmuls (lhsT = [ones | vi] per t-block: PSUM rows 0-63
    accumulate the softmax denominator, 64-127 out^T) accumulate into
    per-half [128, 1024] PSUM tiles, spread across both score phases
    to keep the PE busy exactly while ScalarE chews exps.
  - tail: DVE reciprocal directly on the PSUM denominator, chunked
    multiplies overlapping the output DMA.
"""

import numpy as np
from contextlib import ExitStack

import concourse.bass as bass
import concourse.mybir as mybir
import concourse.tile as tile
from concourse import bacc
from concourse.bass_utils import run_bass_kernel_spmd

E = 768  # n_embd
H = 64  # head size
S = 2048  # sequence length
B = 8  # batch == n_cores
EC = E // 128  # e chunks
TB = S // 128  # t blocks
INV_SQRT_C = float(1.0 / np.sqrt(np.float32(E)))

F16 = mybir.dt.float16
F32 = mybir.dt.float32
F8 = mybir.dt.float8e3  # e3m4: 4 mantissa bits, |x| <= ~15.5

# wpack free-dim layout: [wq 768 | wk 768 | wv 768 | ident 64 | biases 3]
WP_ID = 3 * EC * 128  # 2304
WP_B = WP_ID + 64  # 2368
WP_N = WP_B + 3  # 2371

_CACHE = {}


def build_program():
    nc = bacc.Bacc(
        "TRN2",
        target_bir_lowering=False,
        debug=False,
        enable_asserts=False,
        num_devices=B,
    )

    q_d = nc.dram_tensor("qp", [128, 4, EC, 512], F8, kind="ExternalInput")
    k_d = nc.dram_tensor("kp", [128, 4, EC, 512], F8, kind="ExternalInput")
    v_d = nc.dram_tensor("vp", [128, 4, EC, 512], F16, kind="ExternalInput")
    w_d = nc.dram_tensor("wpack", [128, WP_N], F16, kind="ExternalInput")
    outT_d = nc.dram_tensor("outT", [H, S], F16, kind="ExternalOutput")

    with tile.TileContext(nc) as tc, ExitStack() as ctx:
        const = ctx.enter_context(tc.tile_pool(name="const", bufs=1))
        xin = ctx.enter_context(tc.tile_pool(name="xin", bufs=1))
        acts = ctx.enter_context(tc.tile_pool(name="acts", bufs=1))

        wpack = const.tile([128, WP_N], F16, tag="wpack")
        b32 = const.tile([128, 4], F32, tag="b32")
        warm = const.tile([128, 8], F32, tag="warm")

        q_in = xin.tile([128, 4, EC, 512], F8, tag="q_in")
        k_in = xin.tile([128, 4, EC, 512], F8, tag="k_in")
        v_in = xin.tile([128, 4, EC, 512], F16, tag="v_in")

        # ---- DMA issue: ONE queue (sync), strictly in need-order — the
        # DMA engines round-robin across active queues, so a single ordered
        # queue is the only way to give early-needed transfers full
        # bandwidth.  A single queue's descriptors still spread across all
        # 16 DMA engines.
        nc.sync.dma_start(wpack[:], w_d[:])
        nc.sync.dma_start(k_in[:, 0], k_d[:, 0])
        nc.sync.dma_start(q_in[:, 0], q_d[:, 0])
        nc.sync.dma_start(q_in[:, 1], q_d[:, 1])
        nc.sync.dma_start(k_in[:, 1], k_d[:, 1])
        nc.sync.dma_start(v_in[:, 0], v_d[:, 0])
        nc.sync.dma_start(k_in[:, 2], k_d[:, 2])
        nc.sync.dma_start(q_in[:, 2], q_d[:, 2])
        nc.sync.dma_start(v_in[:, 1], v_d[:, 1])
        nc.sync.dma_start(k_in[:, 3], k_d[:, 3])
        nc.sync.dma_start(q_in[:, 3], q_d[:, 3])
        nc.sync.dma_start(v_in[:, 2], v_d[:, 2])
        nc.sync.dma_start(v_in[:, 3], v_d[:, 3])

        # warm the Exp table on ScalarE while DMAs run
        nc.vector.memset(warm[:], 0.0)
        nc.scalar.activation(
            warm[:], warm[:], mybir.ActivationFunctionType.Exp, scale=1.0
        )
        # biases fp16 -> fp32 scalars (rows 64-127 hold the same values)
        nc.vector.tensor_copy(b32[:, 0:3], wpack[:, WP_B : WP_B + 3])

        qiT = acts.tile([128, S], F16, tag="qiT")
        kiT = acts.tile([128, S], F16, tag="kiT")
        viT = acts.tile([128, S], F16, tag="viT")
        vaug = acts.tile([128, S], F16, tag="vaug")
        recip = acts.tile([H, S], F32, tag="recip")
        out_sb = acts.tile([H, S], F16, tag="out_sb")
        # attention weights, one tile per t-block: cols [h0 1024 | h1 1024]
        attp = ctx.enter_context(tc.tile_pool(name="attp", bufs=16))
        attTs = [
            attp.tile([128, S], F16, tag="attT", name=f"attT{i}") for i in range(TB)
        ]

        # vaug per t-block [128, 128]: cols 0-63 ones (denominator rows),
        # cols 64-127 vi
        nc.vector.memset(vaug[:], 1.0)

        def w_ap(t, c):
            return wpack[:, t * 768 + c * 128 : t * 768 + (c + 1) * 128]

        id_ap = wpack[0:64, WP_ID : WP_ID + 64]

        with tc.tile_pool(name="ps", bufs=2, space="PSUM") as ps, tc.tile_pool(
            name="oph0", bufs=1, space="PSUM"
        ) as oph0:
            po = [oph0.tile([128, 1024], F32, tag="oph0", name="po0"), None]

            def out_mm(tb, h, first=False, last=False):
                # accumulate t-block tb into the s-half h output: rows 0-63
                # denominator, 64-127 out^T.  first/last mark the emission
                # position in the half's accumulation chain.
                for j in range(2):
                    nc.tensor.matmul(
                        po[h][:, j * 512 : (j + 1) * 512],
                        lhsT=vaug[:, tb * 128 : (tb + 1) * 128],
                        rhs=attTs[tb][:, h * 1024 + j * 512 : h * 1024 + (j + 1) * 512],
                        start=first,
                        stop=last,
                    )

            def sc_pair(p, h):
                # two row-tiled concurrent K=64 score matmuls: t-block 2p on
                # PE rows 0-63 -> pa, t-block 2p+1 on rows 64-127 -> pb
                pa = ps.tile([128, 1024], F32, tag="ps", name=f"psA{h}_{p}")
                pb = ps.tile([128, 1024], F32, tag="ps", name=f"psB{h}_{p}")
                for j in range(2):
                    qsl = slice(h * 1024 + j * 512, h * 1024 + (j + 1) * 512)
                    nc.tensor.matmul(
                        pa[:, j * 512 : (j + 1) * 512],
                        lhsT=kiT[0:64, (2 * p) * 128 : (2 * p + 1) * 128],
                        rhs=qiT[0:64, qsl],
                        start=True,
                        stop=True,
                    )
                    nc.tensor.matmul(
                        pb[:, j * 512 : (j + 1) * 512],
                        lhsT=kiT[64:128, (2 * p + 1) * 128 : (2 * p + 2) * 128],
                        rhs=qiT[64:128, qsl],
                        start=True,
                        stop=True,
                    )
                return pa, pb

            def exp_pair(p, h, pab):
                pa, pb = pab
                sl = slice(h * 1024, (h + 1) * 1024)
                nc.scalar.activation(
                    attTs[2 * p][:, sl],
                    pa[:],
                    mybir.ActivationFunctionType.Exp,
                    scale=INV_SQRT_C,
                )
                nc.scalar.activation(
                    attTs[2 * p + 1][:, sl],
                    pb[:],
                    mybir.ActivationFunctionType.Exp,
                    scale=INV_SQRT_C,
                )

            with tc.tile_pool(name="pp", bufs=2, space="PSUM") as pp:

                def proj_qk(j):
                    # col-tiled concurrent pair: q quarter j -> PE cols 0-63
                    # (psum rows 0-63), k quarter j -> cols 64-127
                    pj = pp.tile([128, 512], F32, tag="pp", name=f"pqk{j}")
                    for c in range(EC):
                        nc.tensor.matmul(
                            pj[0:64, :],
                            lhsT=w_ap(0, c)[:, 0:64],
                            rhs=q_in[:, j, c],
                            start=(c == 0),
                            stop=(c == EC - 1),
                            skip_group_check=True,
                        )
                        nc.tensor.matmul(
                            pj[64:128, :],
                            lhsT=w_ap(1, c)[:, 0:64],
                            rhs=k_in[:, j, c],
                            start=(c == 0),
                            stop=(c == EC - 1),
                            skip_group_check=True,
                        )
                    sl = slice(j * 512, (j + 1) * 512)
                    # lo-half adds read PSUM (releases the slot), hi-half
                    # duplicates are cheap fp16 SBUF copies (DVE 4x mode);
                    # late quarters prioritize k (h0 scores need it first)
                    if j >= 2:
                        nc.vector.tensor_scalar_add(
                            kiT[0:64, sl], pj[64:128, :], b32[0:64, 1:2]
                        )
                        nc.vector.tensor_scalar_add(
                            qiT[0:64, sl], pj[0:64, :], b32[0:64, 0:1]
                        )
                        nc.vector.tensor_copy(kiT[64:128, sl], kiT[0:64, sl])
                        nc.vector.tensor_copy(qiT[64:128, sl], qiT[0:64, sl])
                    else:
                        nc.vector.tensor_scalar_add(
                            qiT[0:64, sl], pj[0:64, :], b32[0:64, 0:1]
                        )
                        nc.vector.tensor_scalar_add(
                            kiT[0:64, sl], pj[64:128, :], b32[0:64, 1:2]
                        )
                        nc.vector.tensor_copy(qiT[64:128, sl], qiT[0:64, sl])
                        nc.vector.tensor_copy(kiT[64:128, sl], kiT[0:64, sl])

                def proj_q1():
                    # unpaired q quarter 1 (col tile 0 alone): q1 lands
                    # before k1 and gates the first exp
                    pj = pp.tile([128, 512], F32, tag="pp", name="pq1")
                    for c in range(EC):
                        nc.tensor.matmul(
                            pj[0:64, :],
                            lhsT=w_ap(0, c)[:, 0:64],
                            rhs=q_in[:, 1, c],
                            start=(c == 0),
                            stop=(c == EC - 1),
                        )
                    sl = slice(512, 1024)
                    nc.vector.tensor_scalar_add(qiT[0:64, sl], pj[0:64, :], b32[0:64, 0:1])
                    nc.vector.tensor_copy(qiT[64:128, sl], qiT[0:64, sl])

                def proj_k1():
                    pj = pp.tile([128, 512], F32, tag="pp", name="pk1")
                    for c in range(EC):
                        nc.tensor.matmul(
                            pj[0:64, :],
                            lhsT=w_ap(1, c)[:, 0:64],
                            rhs=k_in[:, 1, c],
                            start=(c == 0),
                            stop=(c == EC - 1),
                        )
                    sl = slice(512, 1024)
                    nc.vector.tensor_scalar_add(kiT[0:64, sl], pj[0:64, :], b32[0:64, 1:2])
                    nc.vector.tensor_copy(kiT[64:128, sl], kiT[0:64, sl])

                def proj_vq(j):
                    # unpaired v quarter j
                    pj = pp.tile([128, 512], F32, tag="pp", name=f"pvq{j}")
                    for c in range(EC):
                        nc.tensor.matmul(
                            pj[0:64, :],
                            lhsT=w_ap(2, c)[:, 0:64],
                            rhs=v_in[:, j, c],
                            start=(c == 0),
                            stop=(c == EC - 1),
                        )
                    nc.vector.tensor_scalar_add(
                        viT[0:64, j * 512 : (j + 1) * 512], pj[0:64, :], b32[0:64, 2:3]
                    )

                def proj_v(jpair):
                    # col-tiled concurrent pair: v quarter 2*jpair -> psum
                    # rows 0-63, v quarter 2*jpair+1 -> rows 64-127
                    j0, j1 = 2 * jpair, 2 * jpair + 1
                    pj = pp.tile([128, 512], F32, tag="pp", name=f"pv{jpair}")
                    for c in range(EC):
                        nc.tensor.matmul(
                            pj[0:64, :],
                            lhsT=w_ap(2, c)[:, 0:64],
                            rhs=v_in[:, j0, c],
                            start=(c == 0),
                            stop=(c == EC - 1),
                            skip_group_check=True,
                        )
                        nc.tensor.matmul(
                            pj[64:128, :],
                            lhsT=w_ap(2, c)[:, 0:64],
                            rhs=v_in[:, j1, c],
                            start=(c == 0),
                            stop=(c == EC - 1),
                            skip_group_check=True,
                        )
                    nc.vector.tensor_scalar_add(
                        viT[0:64, j0 * 512 : (j0 + 1) * 512], pj[0:64, :], b32[0:64, 2:3]
                    )
                    nc.vector.tensor_scalar_add(
                        viT[0:64, j1 * 512 : (j1 + 1) * 512],
                        pj[64:128, :],
                        b32[0:64, 2:3],
                    )

                def transposes():
                    # viT [64, 2048] -> vi blocks [128, 64] into vaug cols
                    # 64-127 via PE transpose
                    for g in range(2):
                        tr = pp.tile([128, 512], F16, tag="pp", name=f"tr{g}")
                        for i in range(8):
                            tb = g * 8 + i
                            nc.tensor.transpose(
                                tr[:, i * 64 : (i + 1) * 64],
                                viT[0:H, tb * 128 : (tb + 1) * 128],
                                id_ap,
                            )
                        dst_ap = vaug[:, g * 1024 : (g + 1) * 1024].rearrange(
                            "p (t c) -> p t c", c=128
                        )[:, :, 64:128]
                        src_ap = tr[:].rearrange("p (t c) -> p t c", c=H)
                        nc.vector.tensor_copy(dst_ap, src_ap)

                # ---- phase 1: projections + h0 scores; fillers placed to
                # match the DMA arrival order (v lands last) ----
                proj_qk(0)
                # pair 0 runs chunked at 512 cols, interleaved with the
                # quarter-1 projections: the first exp only needs q/k
                # quarter 0, so it fires as soon as those are projected
                pa0 = ps.tile([128, 1024], F32, tag="ps", name="psA0_first")
                pb0 = ps.tile([128, 1024], F32, tag="ps", name="psB0_first")

                def sc0(pt, rows, blk, j):
                    qsl = slice(j * 512, (j + 1) * 512)
                    nc.tensor.matmul(
                        pt[:, qsl],
                        lhsT=kiT[rows, blk],
                        rhs=qiT[rows, qsl],
                        start=True,
                        stop=True,
                    )

                def exp0(pt, tb, j):
                    qsl = slice(j * 512, (j + 1) * 512)
                    nc.scalar.activation(
                        attTs[tb][:, qsl],
                        pt[:, qsl],
                        mybir.ActivationFunctionType.Exp,
                        scale=INV_SQRT_C,
                    )

                sc0(pa0, slice(0, 64), slice(0, 128), 0)
                exp0(pa0, 0, 0)
                sc0(pb0, slice(64, 128), slice(128, 256), 0)
                exp0(pb0, 1, 0)
                proj_q1()
                sc0(pa0, slice(0, 64), slice(0, 128), 1)
                exp0(pa0, 0, 1)
                sc0(pb0, slice(64, 128), slice(128, 256), 1)
                exp0(pb0, 1, 1)
                proj_k1()
                for p in range(1, 8):
                    pab = sc_pair(p, 0)
                    if p == 1:
                        proj_vq(0)
                    elif p == 2:
                        proj_qk(2)
                    elif p == 3:
                        proj_vq(1)
                    elif p == 4:
                        proj_qk(3)
                    elif p == 5:
                        proj_vq(2)
                    elif p == 6:
                        proj_vq(3)
                    elif p == 7:
                        transposes()
                    exp_pair(p, 0, pab)

            # ---- phase 2: h1 scores + remaining out accumulation ----
            with tc.tile_pool(name="oph1", bufs=1, space="PSUM") as oph1:
                po[1] = oph1.tile([128, 1024], F32, tag="oph1", name="po1")
                for p in range(8):
                    pab = sc_pair(p, 1)
                    if p == 0:
                        out_mm(0, 0, first=True)
                        out_mm(1, 0)
                        out_mm(2, 0)
                    elif p == 1:
                        out_mm(3, 0)
                        out_mm(4, 0)
                        out_mm(5, 0)
                        out_mm(0, 1, first=True)
                    elif p == 2:
                        out_mm(6, 0)
                        out_mm(7, 0)
                        out_mm(1, 1)
                        out_mm(2, 1)
                    elif p == 3:
                        out_mm(8, 0)
                        out_mm(9, 0)
                        out_mm(3, 1)
                        out_mm(4, 1)
                    elif p == 4:
                        out_mm(10, 0)
                        out_mm(11, 0)
                        out_mm(5, 1)
                        out_mm(6, 1)
                    elif p == 5:
                        out_mm(12, 0)
                        out_mm(13, 0)
                        out_mm(7, 1)
                        out_mm(8, 1)
                    elif p == 6:
                        out_mm(14, 0)
                        out_mm(15, 0, last=True)
                        out_mm(9, 1)
                        out_mm(10, 1)
                    else:
                        out_mm(11, 1)
                        out_mm(12, 1)
                        out_mm(13, 1)
                    if p < 7:
                        exp_pair(p, 1, pab)
                    else:
                        # final pair: chunk the exps and interleave the last
                        # out matmuls so the tail starts sooner
                        pa, pb = pab
                        for j in range(2):
                            sl = slice(1024 + j * 512, 1024 + (j + 1) * 512)
                            nc.scalar.activation(
                                attTs[14][:, sl],
                                pa[:, j * 512 : (j + 1) * 512],
                                mybir.ActivationFunctionType.Exp,
                                scale=INV_SQRT_C,
                            )
                        nc.tensor.matmul(
                            po[1][:, 0:512],
                            lhsT=vaug[:, 14 * 128 : 15 * 128],
                            rhs=attTs[14][:, 1024:1536],
                            start=False,
                            stop=False,
                        )
                        nc.tensor.matmul(
                            po[1][:, 512:1024],
                            lhsT=vaug[:, 14 * 128 : 15 * 128],
                            rhs=attTs[14][:, 1536:2048],
                            start=False,
                            stop=False,
                        )
                        for j in range(2):
                            sl = slice(1024 + j * 512, 1024 + (j + 1) * 512)
                            nc.scalar.activation(
                                attTs[15][:, sl],
                                pb[:, j * 512 : (j + 1) * 512],
                                mybir.ActivationFunctionType.Exp,
                                scale=INV_SQRT_C,
                            )
                        nc.tensor.matmul(
                            po[1][:, 0:512],
                            lhsT=vaug[:, 15 * 128 : 16 * 128],
                            rhs=attTs[15][:, 1024:1536],
                            start=False,
                            stop=True,
                        )
                        nc.tensor.matmul(
                            po[1][:, 512:1024],
                            lhsT=vaug[:, 15 * 128 : 16 * 128],
                            rhs=attTs[15][:, 1536:2048],
                            start=False,
                            stop=True,
                        )

                # ---- tail: per half, denominator (rows 0-63) -> recip ->
                # scale -> DMA out; h0 drains mid-kernel, h1 is chunked so
                # the output DMA starts as early as possible ----
                sl0 = slice(0, 1024)
                nc.vector.reciprocal_approx_fast(recip[:, sl0], po[0][0:64, :])
                nc.vector.tensor_tensor(
                    out_sb[:, sl0], po[0][64:128, :], recip[:, sl0],
                    op=mybir.AluOpType.mult,
                )
                nc.scalar.dma_start(outT_d[:, sl0], out_sb[:, sl0])
                for c in range(2):
                    sl = slice(1024 + c * 512, 1024 + (c + 1) * 512)
                    psl = slice(c * 512, (c + 1) * 512)
                    nc.vector.reciprocal_approx_fast(recip[:, sl], po[1][0:64, psl])
                    nc.vector.tensor_tensor(
                        out_sb[:, sl], po[1][64:128, psl], recip[:, sl],
                        op=mybir.AluOpType.mult,
                    )
                    nc.scalar.dma_start(outT_d[:, sl], out_sb[:, sl])

    nc.compile()
    return nc


def _prep_inputs(q, k, v, Wq, bq, Wk, bk, Wv, bv):
    """Host-side layout prep: per-batch transpose + dtype cast + packing."""
    import ml_dtypes

    wpack = np.zeros((128, WP_N), dtype=np.float16)
    for t, W in enumerate((Wq, Wk, Wv)):
        W2 = np.concatenate([W, W], axis=1)  # [768, 128] duplicated
        wpack[:, t * 768 : (t + 1) * 768] = (
            W2.reshape(EC, 128, 128).transpose(1, 0, 2).reshape(128, 768)
        )
    wpack[0:64, WP_ID : WP_ID + 64] = np.eye(64, dtype=np.float16)
    for i, b in enumerate((bq, bk, bv)):
        wpack[:, WP_B + i] = np.tile(np.asarray(b, dtype=np.float16).reshape(64), 2)

    def pack_x(x, dt):
        # [S, E] -> xT [E, S] -> [128, 4, 6, 512] quarter-major
        xT = np.asarray(x, dtype=dt).T  # [768, 2048]
        return np.ascontiguousarray(xT.reshape(EC, 128, 4, 512).transpose(1, 2, 0, 3))

    f8 = ml_dtypes.float8_e3m4
    in_maps = []
    for i in range(B):
        m = {
            "qp": pack_x(q[i], f8),
            "kp": pack_x(k[i], f8),
            "vp": pack_x(v[i], np.float16),
            "wpack": wpack,
        }
        in_maps.append(m)
    return in_maps


def run(trace=False, **inputs):
    """Build (cached), run on 8 cores, gather. Returns (out, BassKernelResults)."""
    if "nc" not in _CACHE:
        _CACHE["nc"] = build_program()
    nc = _CACHE["nc"]
    in_maps = _prep_inputs(**{k2: np.asarray(v2) for k2, v2 in inputs.items()})
    res = run_bass_kernel_spmd(nc, in_maps, list(range(B)), trace=trace)
    out = np.stack([np.ascontiguousarray(res.results[i]["outT"].T) for i in range(B)])
    return out.astype(np.float32), res


def kernel(**inputs) -> np.ndarray:
    out, _ = run(trace=False, **inputs)
    return out
